# revision 4
# baseline (speedup 1.0000x reference)
# Trainium2 Bass kernel for the powderworld BehaviorFire step.
#
# Full inputs arrive unsharded; batch (B=32) is split 4-per-core across 8
# NeuronCores (pure data parallel — all convs/rolls are per-sample local).
#
# Per-sample layout on chip: each [256,256] plane is stored as an SBUF tile
# [128 partitions, 2 blocks, 256 cols] (partition = row within 128-row block).
# The 3x3 ones conv is separable: the W-direction 3-tap sum runs on the
# vector engine with offset slices; the H-direction 3-tap sum runs on the
# tensor engine as banded matmuls (tridiagonal band + single-element corner
# blocks stitching the two 128-row blocks). The circular H-direction roll
# difference for the velocity update is likewise a banded (circulant) matmul.
# All mask logic is bf16 (values are exact 0/1); velocity math stays f32 and
# follows the reference's operation order so results match bit-for-bit.
import os
import sys

sys.path.insert(0, "/opt/trn_rl_repo")

import numpy as np
import ml_dtypes

def _ensure_axon_hooks():
    # Some images lack antenv.axon_hooks; bass_utils imports it unconditionally
    # when tracing is requested. Provide a minimal shim (hook defaults to None,
    # which makes bass_utils skip tracing gracefully).
    try:
        import antenv.axon_hooks  # noqa: F401
    except ImportError:
        import types

        try:
            import antenv
        except ImportError:
            antenv = types.ModuleType("antenv")
            sys.modules["antenv"] = antenv
        mod = types.ModuleType("antenv.axon_hooks")
        _state = {"hook": None}
        mod.get_axon_ntff_profile_hook = lambda: _state["hook"]
        mod.set_axon_ntff_profile_hook = lambda h: _state.__setitem__("hook", h)
        sys.modules["antenv.axon_hooks"] = mod
        antenv.axon_hooks = mod


_ensure_axon_hooks()

import concourse.bass as bass
import concourse.bacc as bacc
import concourse.mybir as mybir
from concourse.tile import TileContext
from concourse import bass_utils

F32 = mybir.dt.float32
BF16 = mybir.dt.bfloat16
OP = mybir.AluOpType

B, C, H, W = 32, 16, 256, 256
NCORES = 8
B_LOC = B // NCORES
P = 128
KB = 2  # 128-row blocks per plane

EMPTY, WATER, GAS, WOOD, ICE, FIRE, PLANT, LAVA, DUST = 0, 3, 4, 5, 6, 7, 8, 10, 12


def _build_mats() -> np.ndarray:
    """The five 128x128 lhsT matrices used by the tensor engine.

    matmul computes lhsT.T @ rhs, so each entry stores the transpose of the
    block matrix M in out = M @ x."""
    tri = np.zeros((P, P), np.float32)
    for i in range(P):
        for j in (i - 1, i, i + 1):
            if 0 <= j < P:
                tri[i, j] = 1.0
    e_0_127 = np.zeros((P, P), np.float32)
    e_0_127[0, 127] = 1.0
    e_127_0 = np.zeros((P, P), np.float32)
    e_127_0[127, 0] = 1.0
    dy = np.zeros((P, P), np.float32)  # in-block roll diff: R[i] = g[i+1]-g[i-1]
    for i in range(P):
        if i + 1 < P:
            dy[i, i + 1] += 1.0
        if i - 1 >= 0:
            dy[i, i - 1] -= 1.0
    dyx = np.zeros((P, P), np.float32)  # cross-block (and wraparound) part
    dyx[127, 0] = 1.0
    dyx[0, 127] = -1.0
    mats = np.stack(
        [
            tri,      # 0: colsum diagonal block (symmetric)
            e_0_127,  # 1: lhsT of A01 = E(127,0)
            e_127_0,  # 2: lhsT of A10 = E(0,127)
            dy.T,     # 3: lhsT of roll-diff diagonal block
            dyx.T,    # 4: lhsT of roll-diff cross blocks (Dy01 == Dy10)
        ],
        0,
    )
    return mats.astype(ml_dtypes.bfloat16)


def _wsum3(nc, out, x):
    """out[.., j] = x[.., j-1] + x[.., j] + x[.., j+1], zero padded (per block)."""
    v = nc.vector
    v.tensor_tensor(out=out[:, :, 1:255], in0=x[:, :, 0:254], in1=x[:, :, 2:256], op=OP.add)
    v.tensor_tensor(out=out[:, :, 1:255], in0=out[:, :, 1:255], in1=x[:, :, 1:255], op=OP.add)
    v.tensor_tensor(out=out[:, :, 0:1], in0=x[:, :, 0:1], in1=x[:, :, 1:2], op=OP.add)
    v.tensor_tensor(out=out[:, :, 255:256], in0=x[:, :, 254:255], in1=x[:, :, 255:256], op=OP.add)


def _rollx_diff(nc, out, g):
    """out[.., j] = g[.., (j+1)%W] - g[.., (j-1)%W] (circular, per block)."""
    v = nc.vector
    v.tensor_tensor(out=out[:, :, 1:255], in0=g[:, :, 2:256], in1=g[:, :, 0:254], op=OP.subtract)
    v.tensor_tensor(out=out[:, :, 0:1], in0=g[:, :, 1:2], in1=g[:, :, 255:256], op=OP.subtract)
    v.tensor_tensor(out=out[:, :, 255:256], in0=g[:, :, 0:1], in1=g[:, :, 254:255], op=OP.subtract)


def _colsum3(nc, psum, rin, m_tri, m_c01, m_c10):
    """H-direction 3-tap sum (zero padded) of rin into psum, via banded matmuls."""
    nc.tensor.matmul(psum[:, 0, :], m_tri, rin[:, 0, :], start=True, stop=False)
    nc.tensor.matmul(psum[:, 0, :], m_c01, rin[:, 1, :], start=False, stop=True)
    nc.tensor.matmul(psum[:, 1, :], m_tri, rin[:, 1, :], start=True, stop=False)
    nc.tensor.matmul(psum[:, 1, :], m_c10, rin[:, 0, :], start=False, stop=True)


def _rolly_diff(nc, psum, g, m_dy, m_dyx):
    """R[i] = g[(i+1)%H] - g[(i-1)%H] (circular over the full 256 rows)."""
    nc.tensor.matmul(psum[:, 0, :], m_dy, g[:, 0, :], start=True, stop=False)
    nc.tensor.matmul(psum[:, 0, :], m_dyx, g[:, 1, :], start=False, stop=True)
    nc.tensor.matmul(psum[:, 1, :], m_dy, g[:, 1, :], start=True, stop=False)
    nc.tensor.matmul(psum[:, 1, :], m_dyx, g[:, 0, :], start=False, stop=True)


def build_nc(b_loc: int = B_LOC):
    nc = bacc.Bacc("TRN2", target_bir_lowering=False, debug=False)

    world_d = nc.dram_tensor("world", [b_loc, C, H, W], F32, kind="ExternalInput")
    bc_d = nc.dram_tensor("rand_interact", [b_loc, 1, H, W], F32, kind="ExternalInput")
    re_d = nc.dram_tensor("rand_element", [b_loc, 1, H, W], F32, kind="ExternalInput")
    vf_d = nc.dram_tensor("velocity_field", [b_loc, 2, H, W], F32, kind="ExternalInput")
    mats_d = nc.dram_tensor("mats", [5, P, P], BF16, kind="ExternalInput")
    w4_d = nc.dram_tensor("w4", [b_loc, C, H, W], F32, kind="ExternalOutput")
    vfo_d = nc.dram_tensor("vf", [b_loc, 2, H, W], F32, kind="ExternalOutput")

    with TileContext(nc) as tc:
        with (
            tc.tile_pool(name="consts", bufs=1) as cpool,
            tc.tile_pool(name="wpool", bufs=2) as wpool,
            tc.tile_pool(name="io", bufs=2) as iopool,
            tc.tile_pool(name="mk", bufs=2) as mk,
            tc.tile_pool(name="ps", bufs=2, space="PSUM") as ps,
        ):
            mats = cpool.tile([P, 5, P], BF16)
            nc.sync.dma_start(out=mats[:], in_=mats_d[:].rearrange("m k n -> k m n"))
            m_tri = mats[:, 0, :]
            m_c01 = mats[:, 1, :]
            m_c10 = mats[:, 2, :]
            m_dy = mats[:, 3, :]
            m_dyx = mats[:, 4, :]

            v = nc.vector
            g = nc.gpsimd

            for b in range(b_loc):
                # ---- loads ----
                wt = wpool.tile([P, C, KB, W], F32, tag="wt")
                nc.sync.dma_start(
                    out=wt[:], in_=world_d[b].rearrange("c (k p) w -> p c k w", p=P)
                )
                bc = iopool.tile([P, KB, W], F32, tag="bc")
                nc.sync.dma_start(
                    out=bc[:], in_=bc_d[b, 0].rearrange("(k p) w -> p k w", p=P)
                )
                re_ = iopool.tile([P, KB, W], F32, tag="re")
                nc.sync.dma_start(
                    out=re_[:], in_=re_d[b, 0].rearrange("(k p) w -> p k w", p=P)
                )
                vfy = iopool.tile([P, KB, W], F32, tag="vfy")
                nc.sync.dma_start(
                    out=vfy[:], in_=vf_d[b, 0].rearrange("(k p) w -> p k w", p=P)
                )
                vfx = iopool.tile([P, KB, W], F32, tag="vfx")
                nc.sync.dma_start(
                    out=vfx[:], in_=vf_d[b, 1].rearrange("(k p) w -> p k w", p=P)
                )

                ch = lambda c: wt[:, c]

                def mt(tag):
                    return mk.tile([P, KB, W], BF16, tag=tag, name=tag)

                # ---- threshold masks (gpsimd: 1-input ops run at line rate) ----
                bc05 = mt("bc05")
                g.tensor_scalar(out=bc05[:], in0=bc[:], scalar1=0.05, scalar2=None, op0=OP.is_lt)
                bc2 = mt("bc2")
                g.tensor_scalar(out=bc2[:], in0=bc[:], scalar1=0.2, scalar2=None, op0=OP.is_lt)
                bc3 = mt("bc3")
                g.tensor_scalar(out=bc3[:], in0=bc[:], scalar1=0.3, scalar2=None, op0=OP.is_lt)
                re4 = mt("re4")
                g.tensor_scalar(out=re4[:], in0=re_[:], scalar1=0.4, scalar2=None, op0=OP.is_lt)

                # ---- fire neighborhood ----
                fl = mt("fl")
                v.tensor_tensor(out=fl[:], in0=ch(FIRE), in1=ch(LAVA), op=OP.add)
                flr = mt("flr")
                _wsum3(nc, flr, fl)
                flc = ps.tile([P, KB, W], F32, tag="flc")
                _colsum3(nc, flc, flr, m_tri, m_c01, m_c10)
                hfnb = mt("hfnb")
                v.tensor_scalar(out=hfnb[:], in0=flc[:], scalar1=0.0, scalar2=None, op0=OP.is_gt)

                # ---- burn decisions ----
                dbw = mt("dbw")
                v.scalar_tensor_tensor(out=dbw[:], in0=ch(WOOD), scalar=0.5, in1=bc05[:], op0=OP.is_gt, op1=OP.logical_and)
                dbp = mt("dbp")
                v.scalar_tensor_tensor(out=dbp[:], in0=ch(PLANT), scalar=0.5, in1=bc2[:], op0=OP.is_gt, op1=OP.logical_and)
                dbg = mt("dbg")
                v.scalar_tensor_tensor(out=dbg[:], in0=ch(GAS), scalar=0.5, in1=bc2[:], op0=OP.is_gt, op1=OP.logical_and)
                u1 = mt("u1")
                v.tensor_tensor(out=u1[:], in0=dbw[:], in1=dbp[:], op=OP.logical_or)
                u2 = mt("u2")
                v.scalar_tensor_tensor(out=u2[:], in0=ch(DUST), scalar=0.5, in1=dbg[:], op0=OP.is_gt, op1=OP.logical_or)
                ba = mt("ba")
                v.tensor_tensor(out=ba[:], in0=u1[:], in1=u2[:], op=OP.logical_or)
                db = mt("db")
                v.tensor_tensor(out=db[:], in0=ba[:], in1=hfnb[:], op=OP.logical_and)
                md = mt("md")
                v.scalar_tensor_tensor(out=md[:], in0=ch(DUST), scalar=0.5, in1=hfnb[:], op0=OP.is_gt, op1=OP.logical_and)
                tice = mt("tice")
                v.scalar_tensor_tensor(out=tice[:], in0=ch(ICE), scalar=0.5, in1=bc2[:], op0=OP.is_gt, op1=OP.logical_and)
                dbi = mt("dbi")
                v.tensor_tensor(out=dbi[:], in0=tice[:], in1=hfnb[:], op=OP.logical_and)
                r12 = mt("r12")
                v.tensor_tensor(out=r12[:], in0=db[:], in1=dbi[:], op=OP.logical_or)

                # ---- velocity update (order matches the reference: m then md) ----
                ry = ps.tile([P, KB, W], F32, tag="ry")
                _rolly_diff(nc, ry, db, m_dy, m_dyx)
                v.scalar_tensor_tensor(out=vfy[:], in0=ry[:], scalar=-2.0, in1=vfy[:], op0=OP.mult, op1=OP.add)
                ry2 = ps.tile([P, KB, W], F32, tag="ry")
                _rolly_diff(nc, ry2, md, m_dy, m_dyx)
                v.scalar_tensor_tensor(out=vfy[:], in0=ry2[:], scalar=-20.0, in1=vfy[:], op0=OP.mult, op1=OP.add)

                rxm = mt("rxm")
                _rollx_diff(nc, rxm, db)
                v.scalar_tensor_tensor(out=vfx[:], in0=rxm[:], scalar=-2.0, in1=vfx[:], op0=OP.mult, op1=OP.add)
                rxd = mt("rxd")
                _rollx_diff(nc, rxd, md)
                v.scalar_tensor_tensor(out=vfx[:], in0=rxd[:], scalar=-20.0, in1=vfx[:], op0=OP.mult, op1=OP.add)

                # ---- burnables / fire range ----
                s1 = iopool.tile([P, KB, W], F32, tag="s1")
                g.tensor_tensor(out=s1[:], in0=ch(GAS), in1=ch(WOOD), op=OP.add)
                g.tensor_tensor(out=s1[:], in0=s1[:], in1=ch(PLANT), op=OP.add)
                g.tensor_tensor(out=s1[:], in0=s1[:], in1=ch(DUST), op=OP.add)
                burn = mt("burn")
                v.scalar_tensor_tensor(out=burn[:], in0=r12[:], scalar=0.0, in1=s1[:], op0=OP.is_equal, op1=OP.mult)
                lava2 = mt("lava2")
                v.scalar_tensor_tensor(out=lava2[:], in0=r12[:], scalar=0.0, in1=ch(LAVA), op0=OP.is_equal, op1=OP.mult)
                brow = mt("brow")
                _wsum3(nc, brow, burn)
                hbn = ps.tile([P, KB, W], F32, tag="hbn")
                _colsum3(nc, hbn, brow, m_tri, m_c01, m_c10)
                fwbn = mt("fwbn")
                v.tensor_tensor(out=fwbn[:], in0=hbn[:], in1=fl[:], op=OP.mult)
                inner = mt("inner")
                v.tensor_tensor(out=inner[:], in0=fwbn[:], in1=lava2[:], op=OP.add)
                irow = mt("irow")
                _wsum3(nc, irow, inner)
                ifr = ps.tile([P, KB, W], F32, tag="ifr")
                _colsum3(nc, ifr, irow, m_tri, m_c01, m_c10)

                notr12 = mt("notr12")
                g.tensor_scalar(out=notr12[:], in0=r12[:], scalar1=0.0, scalar2=None, op0=OP.is_equal)
                e2 = mt("e2")
                v.scalar_tensor_tensor(out=e2[:], in0=ch(EMPTY), scalar=0.5, in1=notr12[:], op0=OP.is_gt, op1=OP.logical_and)
                ti = mt("ti")
                v.scalar_tensor_tensor(out=ti[:], in0=ifr[:], scalar=0.0, in1=bc3[:], op0=OP.is_gt, op1=OP.logical_and)
                dbe = mt("dbe")
                v.tensor_tensor(out=dbe[:], in0=ti[:], in1=e2[:], op=OP.logical_and)

                nf1 = mt("nf1")
                v.scalar_tensor_tensor(out=nf1[:], in0=ch(FIRE), scalar=0.5, in1=db[:], op0=OP.is_gt, op1=OP.logical_or)
                notdbi = mt("notdbi")
                g.tensor_scalar(out=notdbi[:], in0=dbi[:], scalar1=0.0, scalar2=None, op0=OP.is_equal)
                in2 = mt("in2")
                v.tensor_tensor(out=in2[:], in0=nf1[:], in1=notdbi[:], op=OP.logical_and)
                fire3 = mt("fire3")
                v.tensor_tensor(out=fire3[:], in0=dbe[:], in1=in2[:], op=OP.logical_or)
                z = mt("z")
                v.scalar_tensor_tensor(out=z[:], in0=hbn[:], scalar=0.0, in1=re4[:], op0=OP.is_equal, op1=OP.logical_and)
                dfte = mt("dfte")
                v.tensor_tensor(out=dfte[:], in0=fire3[:], in1=z[:], op=OP.logical_and)

                # ---- output composition ----
                ed = mt("ed")
                v.tensor_tensor(out=ed[:], in0=dfte[:], in1=dbe[:], op=OP.logical_or)
                r_all = mt("r_all")
                v.tensor_tensor(out=r_all[:], in0=ed[:], in1=r12[:], op=OP.logical_or)
                keep = iopool.tile([P, KB, W], F32, tag="keep")
                v.tensor_scalar(out=keep[:], in0=r_all[:], scalar1=0.0, scalar2=None, op0=OP.is_equal)
                mw = mt("mw")
                v.scalar_tensor_tensor(out=mw[:], in0=ed[:], in1=dbi[:], scalar=0.0, op0=OP.is_equal, op1=OP.logical_and)
                bd = mt("bd")
                v.tensor_tensor(out=bd[:], in0=dbi[:], in1=dbe[:], op=OP.logical_or)
                q2 = mt("q2")
                v.scalar_tensor_tensor(out=q2[:], in0=bd[:], scalar=0.0, in1=db[:], op0=OP.is_equal, op1=OP.logical_and)
                q3 = mt("q3")
                v.tensor_tensor(out=q3[:], in0=q2[:], in1=dbe[:], op=OP.logical_or)
                mf = mt("mf")
                v.scalar_tensor_tensor(out=mf[:], in0=dfte[:], scalar=0.0, in1=q3[:], op0=OP.is_equal, op1=OP.logical_and)

                # one big fused pass over all 16 channels (gpsimd, broadcast keep)
                g.tensor_tensor(
                    out=wt[:],
                    in0=wt[:],
                    in1=keep[:, None, :, :].broadcast_to([P, C, KB, W]),
                    op=OP.mult,
                )
                v.tensor_tensor(out=wt[:, EMPTY], in0=wt[:, EMPTY], in1=dfte[:], op=OP.add)
                v.tensor_tensor(out=wt[:, WATER], in0=wt[:, WATER], in1=mw[:], op=OP.add)
                v.tensor_tensor(out=wt[:, FIRE], in0=wt[:, FIRE], in1=mf[:], op=OP.add)

                # ---- stores ----
                nc.sync.dma_start(
                    out=w4_d[b].rearrange("c (k p) w -> p c k w", p=P), in_=wt[:]
                )
                nc.sync.dma_start(
                    out=vfo_d[b, 0].rearrange("(k p) w -> p k w", p=P), in_=vfy[:]
                )
                nc.sync.dma_start(
                    out=vfo_d[b, 1].rearrange("(k p) w -> p k w", p=P), in_=vfx[:]
                )

    return nc


_NC_CACHE = {}


def _get_nc(b_loc: int = B_LOC):
    if b_loc not in _NC_CACHE:
        nc = build_nc(b_loc)
        nc.finalize()
        _NC_CACHE[b_loc] = nc
    return _NC_CACHE[b_loc]


LAST_RESULTS = None


def kernel(**inputs):
    global LAST_RESULTS
    world = np.ascontiguousarray(np.asarray(inputs["world"], dtype=np.float32))
    bc = np.ascontiguousarray(np.asarray(inputs["rand_interact"], dtype=np.float32))
    re_ = np.ascontiguousarray(np.asarray(inputs["rand_element"], dtype=np.float32))
    vf = np.ascontiguousarray(np.asarray(inputs["velocity_field"], dtype=np.float32))
    mats = _build_mats()

    nc = _get_nc()
    in_maps = []
    for i in range(NCORES):
        sl = slice(i * B_LOC, (i + 1) * B_LOC)
        in_maps.append(
            {
                "world": np.ascontiguousarray(world[sl]),
                "rand_interact": np.ascontiguousarray(bc[sl]),
                "rand_element": np.ascontiguousarray(re_[sl]),
                "velocity_field": np.ascontiguousarray(vf[sl]),
                "mats": mats,
            }
        )
    res = bass_utils.run_bass_kernel_spmd(nc, in_maps, core_ids=list(range(NCORES)))
    LAST_RESULTS = res
    w4 = np.concatenate([r["w4"] for r in res.results], 0)
    vfo = np.concatenate([r["vf"] for r in res.results], 0)
    return w4, vfo


# revision 8
# speedup vs baseline: 1.8566x; 1.8566x over previous
# Trainium2 Bass kernel for the powderworld BehaviorFire step.
#
# Full inputs arrive unsharded; batch (B=32) is split 4-per-core across 8
# NeuronCores (pure data parallel — all convs/rolls are per-sample local).
#
# Per-sample layout on chip: each [256,256] plane is stored as an SBUF tile
# [128 partitions, 2 blocks, 256 cols] (partition = row within 128-row block).
# The 3x3 ones conv is separable: the W-direction 3-tap sum runs on the
# vector engine with offset slices; the H-direction 3-tap sum runs on the
# tensor engine as banded matmuls (tridiagonal band + single-element corner
# blocks stitching the two 128-row blocks). The circular H-direction roll
# difference for the velocity update is likewise a banded (circulant) matmul.
# All mask logic is bf16 (values are exact 0/1); velocity math stays f32 and
# follows the reference's operation order so results match bit-for-bit.
import os
import sys

sys.path.insert(0, "/opt/trn_rl_repo")

import numpy as np
import ml_dtypes

def _ensure_axon_hooks():
    # Some images lack antenv.axon_hooks; bass_utils imports it unconditionally
    # when tracing is requested. Provide a minimal shim (hook defaults to None,
    # which makes bass_utils skip tracing gracefully).
    try:
        import antenv.axon_hooks  # noqa: F401
    except ImportError:
        import types

        try:
            import antenv
        except ImportError:
            antenv = types.ModuleType("antenv")
            sys.modules["antenv"] = antenv
        mod = types.ModuleType("antenv.axon_hooks")
        _state = {"hook": None}
        mod.get_axon_ntff_profile_hook = lambda: _state["hook"]
        mod.set_axon_ntff_profile_hook = lambda h: _state.__setitem__("hook", h)
        sys.modules["antenv.axon_hooks"] = mod
        antenv.axon_hooks = mod


_ensure_axon_hooks()

import concourse.bass as bass
import concourse.bacc as bacc
import concourse.mybir as mybir
from concourse.tile import TileContext
from concourse import bass_utils

F32 = mybir.dt.float32
BF16 = mybir.dt.bfloat16
OP = mybir.AluOpType

B, C, H, W = 32, 16, 256, 256
NCORES = 8
B_LOC = B // NCORES
P = 128
KB = 2  # 128-row blocks per plane

EMPTY, WATER, GAS, WOOD, ICE, FIRE, PLANT, LAVA, DUST = 0, 3, 4, 5, 6, 7, 8, 10, 12


def _build_mats() -> np.ndarray:
    """The five 128x128 lhsT matrices used by the tensor engine.

    matmul computes lhsT.T @ rhs, so each entry stores the transpose of the
    block matrix M in out = M @ x."""
    tri = np.zeros((P, P), np.float32)
    for i in range(P):
        for j in (i - 1, i, i + 1):
            if 0 <= j < P:
                tri[i, j] = 1.0
    e_0_127 = np.zeros((P, P), np.float32)
    e_0_127[0, 127] = 1.0
    e_127_0 = np.zeros((P, P), np.float32)
    e_127_0[127, 0] = 1.0
    dy = np.zeros((P, P), np.float32)  # in-block roll diff: R[i] = g[i+1]-g[i-1]
    for i in range(P):
        if i + 1 < P:
            dy[i, i + 1] += 1.0
        if i - 1 >= 0:
            dy[i, i - 1] -= 1.0
    dyx = np.zeros((P, P), np.float32)  # cross-block (and wraparound) part
    dyx[127, 0] = 1.0
    dyx[0, 127] = -1.0
    mats = np.stack(
        [
            tri,      # 0: colsum diagonal block (symmetric)
            e_0_127,  # 1: lhsT of A01 = E(127,0)
            e_127_0,  # 2: lhsT of A10 = E(0,127)
            dy.T,     # 3: lhsT of roll-diff diagonal block
            dyx.T,    # 4: lhsT of roll-diff cross blocks (Dy01 == Dy10)
        ],
        0,
    )
    return mats.astype(ml_dtypes.bfloat16)


def _wsum3(nc, out, x):
    """out[.., j] = x[.., j-1] + x[.., j] + x[.., j+1], zero padded (per block)."""
    v = nc.vector
    v.tensor_tensor(out=out[:, :, 1:255], in0=x[:, :, 0:254], in1=x[:, :, 2:256], op=OP.add)
    v.tensor_tensor(out=out[:, :, 1:255], in0=out[:, :, 1:255], in1=x[:, :, 1:255], op=OP.add)
    v.tensor_tensor(out=out[:, :, 0:1], in0=x[:, :, 0:1], in1=x[:, :, 1:2], op=OP.add)
    v.tensor_tensor(out=out[:, :, 255:256], in0=x[:, :, 254:255], in1=x[:, :, 255:256], op=OP.add)


def _rollx_diff(nc, out, g):
    """out[.., j] = g[.., (j+1)%W] - g[.., (j-1)%W] (circular, per block)."""
    v = nc.vector
    v.tensor_tensor(out=out[:, :, 1:255], in0=g[:, :, 2:256], in1=g[:, :, 0:254], op=OP.subtract)
    v.tensor_tensor(out=out[:, :, 0:1], in0=g[:, :, 1:2], in1=g[:, :, 255:256], op=OP.subtract)
    v.tensor_tensor(out=out[:, :, 255:256], in0=g[:, :, 0:1], in1=g[:, :, 254:255], op=OP.subtract)


def _colsum3(nc, psum, rin, m_tri, m_c01, m_c10):
    """H-direction 3-tap sum (zero padded) of rin into psum, via banded matmuls."""
    nc.tensor.matmul(psum[:, 0, :], m_tri, rin[:, 0, :], start=True, stop=False)
    nc.tensor.matmul(psum[:, 0, :], m_c01, rin[:, 1, :], start=False, stop=True)
    nc.tensor.matmul(psum[:, 1, :], m_tri, rin[:, 1, :], start=True, stop=False)
    nc.tensor.matmul(psum[:, 1, :], m_c10, rin[:, 0, :], start=False, stop=True)


def _rolly_diff(nc, psum, g, m_dy, m_dyx):
    """R[i] = g[(i+1)%H] - g[(i-1)%H] (circular over the full 256 rows)."""
    nc.tensor.matmul(psum[:, 0, :], m_dy, g[:, 0, :], start=True, stop=False)
    nc.tensor.matmul(psum[:, 0, :], m_dyx, g[:, 1, :], start=False, stop=True)
    nc.tensor.matmul(psum[:, 1, :], m_dy, g[:, 1, :], start=True, stop=False)
    nc.tensor.matmul(psum[:, 1, :], m_dyx, g[:, 0, :], start=False, stop=True)


def build_nc(b_loc: int = B_LOC):
    nc = bacc.Bacc("TRN2", target_bir_lowering=False, debug=False)

    world_d = nc.dram_tensor("world", [b_loc, C, H, W], F32, kind="ExternalInput")
    bc_d = nc.dram_tensor("rand_interact", [b_loc, 1, H, W], F32, kind="ExternalInput")
    re_d = nc.dram_tensor("rand_element", [b_loc, 1, H, W], F32, kind="ExternalInput")
    vf_d = nc.dram_tensor("velocity_field", [b_loc, 2, H, W], F32, kind="ExternalInput")
    mats_d = nc.dram_tensor("mats", [5, P, P], BF16, kind="ExternalInput")
    w4_d = nc.dram_tensor("w4", [b_loc, C, H, W], F32, kind="ExternalOutput")
    vfo_d = nc.dram_tensor("vf", [b_loc, 2, H, W], F32, kind="ExternalOutput")

    with TileContext(nc) as tc:
        with (
            tc.tile_pool(name="consts", bufs=1) as cpool,
            tc.tile_pool(name="wpool", bufs=2) as wpool,
            tc.tile_pool(name="io", bufs=2) as iopool,
            tc.tile_pool(name="mk", bufs=2) as mk,
            tc.tile_pool(name="ps", bufs=2, space="PSUM") as ps,
        ):
            mats = cpool.tile([P, 5, P], BF16)
            nc.sync.dma_start(out=mats[:], in_=mats_d[:].rearrange("m k n -> k m n"))
            m_tri = mats[:, 0, :]
            m_c01 = mats[:, 1, :]
            m_c10 = mats[:, 2, :]
            m_dy = mats[:, 3, :]
            m_dyx = mats[:, 4, :]

            v = nc.vector
            g = nc.gpsimd

            for b in range(b_loc):
                # ---- loads ----
                wt = wpool.tile([P, C, KB, W], F32, tag="wt")
                nc.sync.dma_start(
                    out=wt[:], in_=world_d[b].rearrange("c (k p) w -> p c k w", p=P)
                )
                bc = iopool.tile([P, KB, W], F32, tag="bc")
                nc.sync.dma_start(
                    out=bc[:], in_=bc_d[b, 0].rearrange("(k p) w -> p k w", p=P)
                )
                re_ = iopool.tile([P, KB, W], F32, tag="re")
                nc.sync.dma_start(
                    out=re_[:], in_=re_d[b, 0].rearrange("(k p) w -> p k w", p=P)
                )
                vfy = iopool.tile([P, KB, W], F32, tag="vfy")
                nc.sync.dma_start(
                    out=vfy[:], in_=vf_d[b, 0].rearrange("(k p) w -> p k w", p=P)
                )
                vfx = iopool.tile([P, KB, W], F32, tag="vfx")
                nc.sync.dma_start(
                    out=vfx[:], in_=vf_d[b, 1].rearrange("(k p) w -> p k w", p=P)
                )

                ch = lambda c: wt[:, c]

                def mt(tag):
                    return mk.tile([P, KB, W], BF16, tag=tag, name=tag)

                # ---- threshold masks (vector: gpsimd is ~13x slower on ops
                # that convert dtype on write) ----
                bc05 = mt("bc05")
                v.tensor_scalar(out=bc05[:], in0=bc[:], scalar1=0.05, scalar2=None, op0=OP.is_lt)
                bc2 = mt("bc2")
                v.tensor_scalar(out=bc2[:], in0=bc[:], scalar1=0.2, scalar2=None, op0=OP.is_lt)
                bc3 = mt("bc3")
                v.tensor_scalar(out=bc3[:], in0=bc[:], scalar1=0.3, scalar2=None, op0=OP.is_lt)
                re4 = mt("re4")
                v.tensor_scalar(out=re4[:], in0=re_[:], scalar1=0.4, scalar2=None, op0=OP.is_lt)

                # ---- fire neighborhood ----
                fl = mt("fl")
                v.tensor_tensor(out=fl[:], in0=ch(FIRE), in1=ch(LAVA), op=OP.add)
                flr = mt("flr")
                _wsum3(nc, flr, fl)
                flc = ps.tile([P, KB, W], F32, tag="flc")
                _colsum3(nc, flc, flr, m_tri, m_c01, m_c10)
                hfnb = mt("hfnb")
                v.tensor_scalar(out=hfnb[:], in0=flc[:], scalar1=0.0, scalar2=None, op0=OP.is_gt)

                # ---- burn decisions ----
                dbw = mt("dbw")
                v.scalar_tensor_tensor(out=dbw[:], in0=ch(WOOD), scalar=0.5, in1=bc05[:], op0=OP.is_gt, op1=OP.logical_and)
                dbp = mt("dbp")
                v.scalar_tensor_tensor(out=dbp[:], in0=ch(PLANT), scalar=0.5, in1=bc2[:], op0=OP.is_gt, op1=OP.logical_and)
                dbg = mt("dbg")
                v.scalar_tensor_tensor(out=dbg[:], in0=ch(GAS), scalar=0.5, in1=bc2[:], op0=OP.is_gt, op1=OP.logical_and)
                u1 = mt("u1")
                v.tensor_tensor(out=u1[:], in0=dbw[:], in1=dbp[:], op=OP.logical_or)
                u2 = mt("u2")
                v.scalar_tensor_tensor(out=u2[:], in0=ch(DUST), scalar=0.5, in1=dbg[:], op0=OP.is_gt, op1=OP.logical_or)
                ba = mt("ba")
                v.tensor_tensor(out=ba[:], in0=u1[:], in1=u2[:], op=OP.logical_or)
                db = mt("db")
                v.tensor_tensor(out=db[:], in0=ba[:], in1=hfnb[:], op=OP.logical_and)
                md = mt("md")
                v.scalar_tensor_tensor(out=md[:], in0=ch(DUST), scalar=0.5, in1=hfnb[:], op0=OP.is_gt, op1=OP.logical_and)
                tice = mt("tice")
                v.scalar_tensor_tensor(out=tice[:], in0=ch(ICE), scalar=0.5, in1=bc2[:], op0=OP.is_gt, op1=OP.logical_and)
                dbi = mt("dbi")
                v.tensor_tensor(out=dbi[:], in0=tice[:], in1=hfnb[:], op=OP.logical_and)
                r12 = mt("r12")
                v.tensor_tensor(out=r12[:], in0=db[:], in1=dbi[:], op=OP.logical_or)

                # ---- velocity update (order matches the reference: m then md) ----
                ry = ps.tile([P, KB, W], F32, tag="ry")
                _rolly_diff(nc, ry, db, m_dy, m_dyx)
                v.scalar_tensor_tensor(out=vfy[:], in0=ry[:], scalar=-2.0, in1=vfy[:], op0=OP.mult, op1=OP.add)
                ry2 = ps.tile([P, KB, W], F32, tag="ry")
                _rolly_diff(nc, ry2, md, m_dy, m_dyx)
                v.scalar_tensor_tensor(out=vfy[:], in0=ry2[:], scalar=-20.0, in1=vfy[:], op0=OP.mult, op1=OP.add)

                rxm = mt("rxm")
                _rollx_diff(nc, rxm, db)
                v.scalar_tensor_tensor(out=vfx[:], in0=rxm[:], scalar=-2.0, in1=vfx[:], op0=OP.mult, op1=OP.add)
                rxd = mt("rxd")
                _rollx_diff(nc, rxd, md)
                v.scalar_tensor_tensor(out=vfx[:], in0=rxd[:], scalar=-20.0, in1=vfx[:], op0=OP.mult, op1=OP.add)

                # ---- burnables / fire range ----
                s1 = iopool.tile([P, KB, W], F32, tag="s1")
                g.tensor_tensor(out=s1[:], in0=ch(GAS), in1=ch(WOOD), op=OP.add)
                g.tensor_tensor(out=s1[:], in0=s1[:], in1=ch(PLANT), op=OP.add)
                g.tensor_tensor(out=s1[:], in0=s1[:], in1=ch(DUST), op=OP.add)
                burn = mt("burn")
                v.scalar_tensor_tensor(out=burn[:], in0=r12[:], scalar=0.0, in1=s1[:], op0=OP.is_equal, op1=OP.mult)
                lava2 = mt("lava2")
                v.scalar_tensor_tensor(out=lava2[:], in0=r12[:], scalar=0.0, in1=ch(LAVA), op0=OP.is_equal, op1=OP.mult)
                brow = mt("brow")
                _wsum3(nc, brow, burn)
                hbn = ps.tile([P, KB, W], F32, tag="hbn")
                _colsum3(nc, hbn, brow, m_tri, m_c01, m_c10)
                fwbn = mt("fwbn")
                v.tensor_tensor(out=fwbn[:], in0=hbn[:], in1=fl[:], op=OP.mult)
                inner = mt("inner")
                v.tensor_tensor(out=inner[:], in0=fwbn[:], in1=lava2[:], op=OP.add)
                irow = mt("irow")
                _wsum3(nc, irow, inner)
                ifr = ps.tile([P, KB, W], F32, tag="ifr")
                _colsum3(nc, ifr, irow, m_tri, m_c01, m_c10)

                notr12 = mt("notr12")
                v.tensor_scalar(out=notr12[:], in0=r12[:], scalar1=0.0, scalar2=None, op0=OP.is_equal)
                e2 = mt("e2")
                v.scalar_tensor_tensor(out=e2[:], in0=ch(EMPTY), scalar=0.5, in1=notr12[:], op0=OP.is_gt, op1=OP.logical_and)
                ti = mt("ti")
                v.scalar_tensor_tensor(out=ti[:], in0=ifr[:], scalar=0.0, in1=bc3[:], op0=OP.is_gt, op1=OP.logical_and)
                dbe = mt("dbe")
                v.tensor_tensor(out=dbe[:], in0=ti[:], in1=e2[:], op=OP.logical_and)

                nf1 = mt("nf1")
                v.scalar_tensor_tensor(out=nf1[:], in0=ch(FIRE), scalar=0.5, in1=db[:], op0=OP.is_gt, op1=OP.logical_or)
                notdbi = mt("notdbi")
                v.tensor_scalar(out=notdbi[:], in0=dbi[:], scalar1=0.0, scalar2=None, op0=OP.is_equal)
                in2 = mt("in2")
                v.tensor_tensor(out=in2[:], in0=nf1[:], in1=notdbi[:], op=OP.logical_and)
                fire3 = mt("fire3")
                v.tensor_tensor(out=fire3[:], in0=dbe[:], in1=in2[:], op=OP.logical_or)
                z = mt("z")
                v.scalar_tensor_tensor(out=z[:], in0=hbn[:], scalar=0.0, in1=re4[:], op0=OP.is_equal, op1=OP.logical_and)
                dfte = mt("dfte")
                v.tensor_tensor(out=dfte[:], in0=fire3[:], in1=z[:], op=OP.logical_and)

                # ---- output composition ----
                ed = mt("ed")
                v.tensor_tensor(out=ed[:], in0=dfte[:], in1=dbe[:], op=OP.logical_or)
                r_all = mt("r_all")
                v.tensor_tensor(out=r_all[:], in0=ed[:], in1=r12[:], op=OP.logical_or)
                keep = iopool.tile([P, KB, W], F32, tag="keep")
                v.tensor_scalar(out=keep[:], in0=r_all[:], scalar1=0.0, scalar2=None, op0=OP.is_equal)
                mw = mt("mw")
                v.scalar_tensor_tensor(out=mw[:], in0=ed[:], in1=dbi[:], scalar=0.0, op0=OP.is_equal, op1=OP.logical_and)
                bd = mt("bd")
                v.tensor_tensor(out=bd[:], in0=dbi[:], in1=dbe[:], op=OP.logical_or)
                q2 = mt("q2")
                v.scalar_tensor_tensor(out=q2[:], in0=bd[:], scalar=0.0, in1=db[:], op0=OP.is_equal, op1=OP.logical_and)
                q3 = mt("q3")
                v.tensor_tensor(out=q3[:], in0=q2[:], in1=dbe[:], op=OP.logical_or)
                mf = mt("mf")
                v.scalar_tensor_tensor(out=mf[:], in0=dfte[:], scalar=0.0, in1=q3[:], op0=OP.is_equal, op1=OP.logical_and)

                # fused keep-multiply over all 16 channels, split across engines:
                # gpsimd takes 7 pure-passthrough channels (f32 in/out only —
                # no dtype conversion, where gpsimd is acceptable), vector the
                # other 9 (incl. 0/3/7 so their +mask adds can start early).
                v.tensor_tensor(
                    out=wt[:, 0:9],
                    in0=wt[:, 0:9],
                    in1=keep[:, None, :, :].broadcast_to([P, 9, KB, W]),
                    op=OP.mult,
                )
                g.tensor_tensor(
                    out=wt[:, 9:16],
                    in0=wt[:, 9:16],
                    in1=keep[:, None, :, :].broadcast_to([P, 7, KB, W]),
                    op=OP.mult,
                )
                v.tensor_tensor(out=wt[:, EMPTY], in0=wt[:, EMPTY], in1=dfte[:], op=OP.add)
                v.tensor_tensor(out=wt[:, WATER], in0=wt[:, WATER], in1=mw[:], op=OP.add)
                v.tensor_tensor(out=wt[:, FIRE], in0=wt[:, FIRE], in1=mf[:], op=OP.add)

                # ---- stores ----
                nc.sync.dma_start(
                    out=w4_d[b].rearrange("c (k p) w -> p c k w", p=P), in_=wt[:]
                )
                nc.sync.dma_start(
                    out=vfo_d[b, 0].rearrange("(k p) w -> p k w", p=P), in_=vfy[:]
                )
                nc.sync.dma_start(
                    out=vfo_d[b, 1].rearrange("(k p) w -> p k w", p=P), in_=vfx[:]
                )

    return nc


_NC_CACHE = {}


def _get_nc(b_loc: int = B_LOC):
    if b_loc not in _NC_CACHE:
        nc = build_nc(b_loc)
        nc.finalize()
        _NC_CACHE[b_loc] = nc
    return _NC_CACHE[b_loc]


LAST_RESULTS = None


def kernel(**inputs):
    global LAST_RESULTS
    world = np.ascontiguousarray(np.asarray(inputs["world"], dtype=np.float32))
    bc = np.ascontiguousarray(np.asarray(inputs["rand_interact"], dtype=np.float32))
    re_ = np.ascontiguousarray(np.asarray(inputs["rand_element"], dtype=np.float32))
    vf = np.ascontiguousarray(np.asarray(inputs["velocity_field"], dtype=np.float32))
    mats = _build_mats()

    nc = _get_nc()
    in_maps = []
    for i in range(NCORES):
        sl = slice(i * B_LOC, (i + 1) * B_LOC)
        in_maps.append(
            {
                "world": np.ascontiguousarray(world[sl]),
                "rand_interact": np.ascontiguousarray(bc[sl]),
                "rand_element": np.ascontiguousarray(re_[sl]),
                "velocity_field": np.ascontiguousarray(vf[sl]),
                "mats": mats,
            }
        )
    res = bass_utils.run_bass_kernel_spmd(nc, in_maps, core_ids=list(range(NCORES)))
    LAST_RESULTS = res
    w4 = np.concatenate([r["w4"] for r in res.results], 0)
    vfo = np.concatenate([r["vf"] for r in res.results], 0)
    return w4, vfo


# revision 9
# speedup vs baseline: 2.4932x; 1.3429x over previous
# Trainium2 Bass kernel for the powderworld BehaviorFire step.
#
# Full inputs arrive unsharded; batch (B=32) is split 4-per-core across 8
# NeuronCores (pure data parallel — all convs/rolls are per-sample local).
#
# Per-sample layout on chip: each [256,256] plane is stored as an SBUF tile
# [128 partitions, 2 blocks, 256 cols] (partition = row within 128-row block).
# The 3x3 ones conv is separable: the W-direction 3-tap sum runs on the
# vector engine with offset slices; the H-direction 3-tap sum runs on the
# tensor engine as banded matmuls (tridiagonal band + single-element corner
# blocks stitching the two 128-row blocks). The circular H-direction roll
# difference for the velocity update is likewise a banded (circulant) matmul.
#
# The world tensor is one-hot (values exactly 0.0/1.0), so it is carried in
# bf16 end-to-end (exact) — halving both its DMA traffic and the vector
# engine's element throughput cost. The host casts f32->bf16 on the way in
# and bf16->f32 on the way out; a runtime check falls back to an all-f32
# kernel if the input world is ever not bf16-exact. Velocity math stays f32
# and follows the reference's operation order bit-for-bit. The burnable /
# fire-range convolutions only feed >0 / ==0 tests downstream, so their
# inputs are replaced by sign()-proxies with identical zero sets, computed
# on the otherwise-idle scalar (ACT) engine.
import os
import sys

sys.path.insert(0, "/opt/trn_rl_repo")

import numpy as np
import ml_dtypes


def _ensure_axon_hooks():
    # Some images lack antenv.axon_hooks; bass_utils imports it unconditionally
    # when tracing is requested. Provide a minimal shim (hook defaults to None,
    # which makes bass_utils skip tracing gracefully).
    try:
        import antenv.axon_hooks  # noqa: F401
    except ImportError:
        import types

        try:
            import antenv
        except ImportError:
            antenv = types.ModuleType("antenv")
            sys.modules["antenv"] = antenv
        mod = types.ModuleType("antenv.axon_hooks")
        _state = {"hook": None}
        mod.get_axon_ntff_profile_hook = lambda: _state["hook"]
        mod.set_axon_ntff_profile_hook = lambda h: _state.__setitem__("hook", h)
        sys.modules["antenv.axon_hooks"] = mod
        antenv.axon_hooks = mod


_ensure_axon_hooks()

import concourse.bass as bass
import concourse.bacc as bacc
import concourse.mybir as mybir
from concourse.tile import TileContext
from concourse import bass_utils

F32 = mybir.dt.float32
BF16 = mybir.dt.bfloat16
OP = mybir.AluOpType
AF = mybir.ActivationFunctionType

B, C, H, W = 32, 16, 256, 256
NCORES = 8
B_LOC = B // NCORES
P = 128
KB = 2  # 128-row blocks per plane

EMPTY, WATER, GAS, WOOD, ICE, FIRE, PLANT, LAVA, DUST = 0, 3, 4, 5, 6, 7, 8, 10, 12


def _build_mats() -> np.ndarray:
    """The five 128x128 lhsT matrices used by the tensor engine.

    matmul computes lhsT.T @ rhs, so each entry stores the transpose of the
    block matrix M in out = M @ x."""
    tri = np.zeros((P, P), np.float32)
    for i in range(P):
        for j in (i - 1, i, i + 1):
            if 0 <= j < P:
                tri[i, j] = 1.0
    e_0_127 = np.zeros((P, P), np.float32)
    e_0_127[0, 127] = 1.0
    e_127_0 = np.zeros((P, P), np.float32)
    e_127_0[127, 0] = 1.0
    dy = np.zeros((P, P), np.float32)  # in-block roll diff: R[i] = g[i+1]-g[i-1]
    for i in range(P):
        if i + 1 < P:
            dy[i, i + 1] += 1.0
        if i - 1 >= 0:
            dy[i, i - 1] -= 1.0
    dyx = np.zeros((P, P), np.float32)  # cross-block (and wraparound) part
    dyx[127, 0] = 1.0
    dyx[0, 127] = -1.0
    mats = np.stack(
        [
            tri,      # 0: colsum diagonal block (symmetric)
            e_0_127,  # 1: lhsT of A01 = E(127,0)
            e_127_0,  # 2: lhsT of A10 = E(0,127)
            dy.T,     # 3: lhsT of roll-diff diagonal block
            dyx.T,    # 4: lhsT of roll-diff cross blocks (Dy01 == Dy10)
        ],
        0,
    )
    return mats.astype(ml_dtypes.bfloat16)


def _wsum3(nc, out, x):
    """out[.., j] = x[.., j-1] + x[.., j] + x[.., j+1], zero padded (per block)."""
    v = nc.vector
    v.tensor_tensor(out=out[:, :, 1:255], in0=x[:, :, 0:254], in1=x[:, :, 2:256], op=OP.add)
    v.tensor_tensor(out=out[:, :, 1:255], in0=out[:, :, 1:255], in1=x[:, :, 1:255], op=OP.add)
    v.tensor_tensor(out=out[:, :, 0:1], in0=x[:, :, 0:1], in1=x[:, :, 1:2], op=OP.add)
    v.tensor_tensor(out=out[:, :, 255:256], in0=x[:, :, 254:255], in1=x[:, :, 255:256], op=OP.add)


def _rollx_diff(nc, out, g):
    """out[.., j] = g[.., (j+1)%W] - g[.., (j-1)%W] (circular, per block)."""
    v = nc.vector
    v.tensor_tensor(out=out[:, :, 1:255], in0=g[:, :, 2:256], in1=g[:, :, 0:254], op=OP.subtract)
    v.tensor_tensor(out=out[:, :, 0:1], in0=g[:, :, 1:2], in1=g[:, :, 255:256], op=OP.subtract)
    v.tensor_tensor(out=out[:, :, 255:256], in0=g[:, :, 0:1], in1=g[:, :, 254:255], op=OP.subtract)


def _colsum3(nc, psum, rin, m_tri, m_c01, m_c10):
    """H-direction 3-tap sum (zero padded) of rin into psum, via banded matmuls."""
    nc.tensor.matmul(psum[:, 0, :], m_tri, rin[:, 0, :], start=True, stop=False)
    nc.tensor.matmul(psum[:, 0, :], m_c01, rin[:, 1, :], start=False, stop=True)
    nc.tensor.matmul(psum[:, 1, :], m_tri, rin[:, 1, :], start=True, stop=False)
    nc.tensor.matmul(psum[:, 1, :], m_c10, rin[:, 0, :], start=False, stop=True)


def _rolly_diff(nc, psum, g, m_dy, m_dyx):
    """R[i] = g[(i+1)%H] - g[(i-1)%H] (circular over the full 256 rows)."""
    nc.tensor.matmul(psum[:, 0, :], m_dy, g[:, 0, :], start=True, stop=False)
    nc.tensor.matmul(psum[:, 0, :], m_dyx, g[:, 1, :], start=False, stop=True)
    nc.tensor.matmul(psum[:, 1, :], m_dy, g[:, 1, :], start=True, stop=False)
    nc.tensor.matmul(psum[:, 1, :], m_dyx, g[:, 0, :], start=False, stop=True)


def build_nc(b_loc: int = B_LOC, world_bf16: bool = True):
    WDT = BF16 if world_bf16 else F32
    nc = bacc.Bacc("TRN2", target_bir_lowering=False, debug=False)

    world_d = nc.dram_tensor("world", [b_loc, C, H, W], WDT, kind="ExternalInput")
    bc_d = nc.dram_tensor("rand_interact", [b_loc, 1, H, W], F32, kind="ExternalInput")
    re_d = nc.dram_tensor("rand_element", [b_loc, 1, H, W], F32, kind="ExternalInput")
    vf_d = nc.dram_tensor("velocity_field", [b_loc, 2, H, W], F32, kind="ExternalInput")
    mats_d = nc.dram_tensor("mats", [5, P, P], BF16, kind="ExternalInput")
    w4_d = nc.dram_tensor("w4", [b_loc, C, H, W], WDT, kind="ExternalOutput")
    vfo_d = nc.dram_tensor("vf", [b_loc, 2, H, W], F32, kind="ExternalOutput")

    with TileContext(nc) as tc:
        with (
            tc.tile_pool(name="consts", bufs=1) as cpool,
            tc.tile_pool(name="wpool", bufs=2) as wpool,
            tc.tile_pool(name="io", bufs=3) as iopool,
            tc.tile_pool(name="mk", bufs=2) as mk,
            tc.tile_pool(name="ps", bufs=2, space="PSUM") as ps,
        ):
            mats = cpool.tile([P, 5, P], BF16)
            nc.sync.dma_start(out=mats[:], in_=mats_d[:].rearrange("m k n -> k m n"))
            m_tri = mats[:, 0, :]
            m_c01 = mats[:, 1, :]
            m_c10 = mats[:, 2, :]
            m_dy = mats[:, 3, :]
            m_dyx = mats[:, 4, :]

            v = nc.vector
            act = nc.scalar

            for b in range(b_loc):
                # ---- loads ----
                wt = wpool.tile([P, C, KB, W], WDT, tag="wt")
                nc.sync.dma_start(
                    out=wt[:], in_=world_d[b].rearrange("c (k p) w -> p c k w", p=P)
                )
                bc = iopool.tile([P, KB, W], F32, tag="bc")
                nc.sync.dma_start(
                    out=bc[:], in_=bc_d[b, 0].rearrange("(k p) w -> p k w", p=P)
                )
                re_ = iopool.tile([P, KB, W], F32, tag="re")
                nc.sync.dma_start(
                    out=re_[:], in_=re_d[b, 0].rearrange("(k p) w -> p k w", p=P)
                )
                vfy = iopool.tile([P, KB, W], F32, tag="vfy")
                nc.sync.dma_start(
                    out=vfy[:], in_=vf_d[b, 0].rearrange("(k p) w -> p k w", p=P)
                )
                vfx = iopool.tile([P, KB, W], F32, tag="vfx")
                nc.sync.dma_start(
                    out=vfx[:], in_=vf_d[b, 1].rearrange("(k p) w -> p k w", p=P)
                )

                ch = lambda c: wt[:, c]

                def mt(tag):
                    return mk.tile([P, KB, W], BF16, tag=tag, name=tag)

                # ---- threshold masks ----
                bc05 = mt("bc05")
                v.tensor_scalar(out=bc05[:], in0=bc[:], scalar1=0.05, scalar2=None, op0=OP.is_lt)
                bc2 = mt("bc2")
                v.tensor_scalar(out=bc2[:], in0=bc[:], scalar1=0.2, scalar2=None, op0=OP.is_lt)
                bc3 = mt("bc3")
                v.tensor_scalar(out=bc3[:], in0=bc[:], scalar1=0.3, scalar2=None, op0=OP.is_lt)
                re4 = mt("re4")
                v.tensor_scalar(out=re4[:], in0=re_[:], scalar1=0.4, scalar2=None, op0=OP.is_lt)

                # ---- fire neighborhood ----
                fl = mt("fl")
                v.tensor_tensor(out=fl[:], in0=ch(FIRE), in1=ch(LAVA), op=OP.add)
                flr = mt("flr")
                _wsum3(nc, flr, fl)
                flc = ps.tile([P, KB, W], F32, tag="flc")
                _colsum3(nc, flc, flr, m_tri, m_c01, m_c10)
                # conv >0 on non-negative ints == sign() — runs on idle ACT engine
                hfnb = mt("hfnb")
                act.sign(out=hfnb[:], in_=flc[:])

                # ---- burn decisions ----
                dbw = mt("dbw")
                v.scalar_tensor_tensor(out=dbw[:], in0=ch(WOOD), scalar=0.5, in1=bc05[:], op0=OP.is_gt, op1=OP.logical_and)
                dbp = mt("dbp")
                v.scalar_tensor_tensor(out=dbp[:], in0=ch(PLANT), scalar=0.5, in1=bc2[:], op0=OP.is_gt, op1=OP.logical_and)
                dbg = mt("dbg")
                v.scalar_tensor_tensor(out=dbg[:], in0=ch(GAS), scalar=0.5, in1=bc2[:], op0=OP.is_gt, op1=OP.logical_and)
                u1 = mt("u1")
                v.tensor_tensor(out=u1[:], in0=dbw[:], in1=dbp[:], op=OP.logical_or)
                u2 = mt("u2")
                v.scalar_tensor_tensor(out=u2[:], in0=ch(DUST), scalar=0.5, in1=dbg[:], op0=OP.is_gt, op1=OP.logical_or)
                ba = mt("ba")
                v.tensor_tensor(out=ba[:], in0=u1[:], in1=u2[:], op=OP.logical_or)
                db = mt("db")
                v.tensor_tensor(out=db[:], in0=ba[:], in1=hfnb[:], op=OP.logical_and)
                md = mt("md")
                v.scalar_tensor_tensor(out=md[:], in0=ch(DUST), scalar=0.5, in1=hfnb[:], op0=OP.is_gt, op1=OP.logical_and)
                tice = mt("tice")
                v.scalar_tensor_tensor(out=tice[:], in0=ch(ICE), scalar=0.5, in1=bc2[:], op0=OP.is_gt, op1=OP.logical_and)
                dbi = mt("dbi")
                v.tensor_tensor(out=dbi[:], in0=tice[:], in1=hfnb[:], op=OP.logical_and)
                r12 = mt("r12")
                v.tensor_tensor(out=r12[:], in0=db[:], in1=dbi[:], op=OP.logical_or)

                # ---- velocity update (order matches the reference: m then md) ----
                ry = ps.tile([P, KB, W], F32, tag="ry")
                _rolly_diff(nc, ry, db, m_dy, m_dyx)
                v.scalar_tensor_tensor(out=vfy[:], in0=ry[:], scalar=-2.0, in1=vfy[:], op0=OP.mult, op1=OP.add)
                ry2 = ps.tile([P, KB, W], F32, tag="ry")
                _rolly_diff(nc, ry2, md, m_dy, m_dyx)
                v.scalar_tensor_tensor(out=vfy[:], in0=ry2[:], scalar=-20.0, in1=vfy[:], op0=OP.mult, op1=OP.add)

                rxm = mt("rxm")
                _rollx_diff(nc, rxm, db)
                v.scalar_tensor_tensor(out=vfx[:], in0=rxm[:], scalar=-2.0, in1=vfx[:], op0=OP.mult, op1=OP.add)
                rxd = mt("rxd")
                _rollx_diff(nc, rxd, md)
                v.scalar_tensor_tensor(out=vfx[:], in0=rxd[:], scalar=-20.0, in1=vfx[:], op0=OP.mult, op1=OP.add)

                # ---- burnables / fire range ----
                # Downstream only tests conv results for >0 / ==0, so sign()
                # proxies (same zero set, values in {0,1,2}) are exact here.
                s1 = mk.tile([P, KB, W], BF16, tag="s1", name="s1")
                v.tensor_tensor(out=s1[:], in0=ch(GAS), in1=ch(WOOD), op=OP.add)
                v.tensor_tensor(out=s1[:], in0=s1[:], in1=ch(PLANT), op=OP.add)
                v.tensor_tensor(out=s1[:], in0=s1[:], in1=ch(DUST), op=OP.add)
                burn = mt("burn")
                v.scalar_tensor_tensor(out=burn[:], in0=r12[:], scalar=0.0, in1=s1[:], op0=OP.is_equal, op1=OP.mult)
                lava2 = mt("lava2")
                v.scalar_tensor_tensor(out=lava2[:], in0=r12[:], scalar=0.0, in1=ch(LAVA), op0=OP.is_equal, op1=OP.mult)
                brow = mt("brow")
                _wsum3(nc, brow, burn)
                hbn = ps.tile([P, KB, W], F32, tag="hbn")
                _colsum3(nc, hbn, brow, m_tri, m_c01, m_c10)
                sghbn = mt("sghbn")
                act.sign(out=sghbn[:], in_=hbn[:])
                fwbn = mt("fwbn")
                v.tensor_tensor(out=fwbn[:], in0=sghbn[:], in1=fl[:], op=OP.mult)
                inner = mt("inner")
                v.tensor_tensor(out=inner[:], in0=fwbn[:], in1=lava2[:], op=OP.add)
                irow = mt("irow")
                _wsum3(nc, irow, inner)
                ifr = ps.tile([P, KB, W], F32, tag="ifr")
                _colsum3(nc, ifr, irow, m_tri, m_c01, m_c10)
                sgifr = mt("sgifr")
                act.sign(out=sgifr[:], in_=ifr[:])

                notr12 = mt("notr12")
                v.tensor_scalar(out=notr12[:], in0=r12[:], scalar1=0.0, scalar2=None, op0=OP.is_equal)
                e2 = mt("e2")
                v.scalar_tensor_tensor(out=e2[:], in0=ch(EMPTY), scalar=0.5, in1=notr12[:], op0=OP.is_gt, op1=OP.logical_and)
                ti = mt("ti")
                v.tensor_tensor(out=ti[:], in0=sgifr[:], in1=bc3[:], op=OP.logical_and)
                dbe = mt("dbe")
                v.tensor_tensor(out=dbe[:], in0=ti[:], in1=e2[:], op=OP.logical_and)

                nf1 = mt("nf1")
                v.scalar_tensor_tensor(out=nf1[:], in0=ch(FIRE), scalar=0.5, in1=db[:], op0=OP.is_gt, op1=OP.logical_or)
                notdbi = mt("notdbi")
                v.tensor_scalar(out=notdbi[:], in0=dbi[:], scalar1=0.0, scalar2=None, op0=OP.is_equal)
                in2 = mt("in2")
                v.tensor_tensor(out=in2[:], in0=nf1[:], in1=notdbi[:], op=OP.logical_and)
                fire3 = mt("fire3")
                v.tensor_tensor(out=fire3[:], in0=dbe[:], in1=in2[:], op=OP.logical_or)
                z = mt("z")
                v.scalar_tensor_tensor(out=z[:], in0=sghbn[:], scalar=0.0, in1=re4[:], op0=OP.is_equal, op1=OP.logical_and)
                dfte = mt("dfte")
                v.tensor_tensor(out=dfte[:], in0=fire3[:], in1=z[:], op=OP.logical_and)

                # ---- output composition ----
                ed = mt("ed")
                v.tensor_tensor(out=ed[:], in0=dfte[:], in1=dbe[:], op=OP.logical_or)
                r_all = mt("r_all")
                v.tensor_tensor(out=r_all[:], in0=ed[:], in1=r12[:], op=OP.logical_or)
                keep = mt("keep")
                v.tensor_scalar(out=keep[:], in0=r_all[:], scalar1=0.0, scalar2=None, op0=OP.is_equal)
                mw = mt("mw")
                v.scalar_tensor_tensor(out=mw[:], in0=ed[:], in1=dbi[:], scalar=0.0, op0=OP.is_equal, op1=OP.logical_and)
                bd = mt("bd")
                v.tensor_tensor(out=bd[:], in0=dbi[:], in1=dbe[:], op=OP.logical_or)
                q2 = mt("q2")
                v.scalar_tensor_tensor(out=q2[:], in0=bd[:], scalar=0.0, in1=db[:], op0=OP.is_equal, op1=OP.logical_and)
                q3 = mt("q3")
                v.tensor_tensor(out=q3[:], in0=q2[:], in1=dbe[:], op=OP.logical_or)
                mf = mt("mf")
                v.scalar_tensor_tensor(out=mf[:], in0=dfte[:], scalar=0.0, in1=q3[:], op0=OP.is_equal, op1=OP.logical_and)

                # fused keep-multiply over all 16 channels, then the 3 one-hot adds
                v.tensor_tensor(
                    out=wt[:],
                    in0=wt[:],
                    in1=keep[:, None, :, :].broadcast_to([P, C, KB, W]),
                    op=OP.mult,
                )
                v.tensor_tensor(out=wt[:, EMPTY], in0=wt[:, EMPTY], in1=dfte[:], op=OP.add)
                v.tensor_tensor(out=wt[:, WATER], in0=wt[:, WATER], in1=mw[:], op=OP.add)
                v.tensor_tensor(out=wt[:, FIRE], in0=wt[:, FIRE], in1=mf[:], op=OP.add)

                # ---- stores ----
                nc.sync.dma_start(
                    out=w4_d[b].rearrange("c (k p) w -> p c k w", p=P), in_=wt[:]
                )
                nc.sync.dma_start(
                    out=vfo_d[b, 0].rearrange("(k p) w -> p k w", p=P), in_=vfy[:]
                )
                nc.sync.dma_start(
                    out=vfo_d[b, 1].rearrange("(k p) w -> p k w", p=P), in_=vfx[:]
                )

    return nc


_NC_CACHE = {}


def _get_nc(b_loc: int = B_LOC, world_bf16: bool = True):
    key = (b_loc, world_bf16)
    if key not in _NC_CACHE:
        nc = build_nc(b_loc, world_bf16)
        nc.finalize()
        _NC_CACHE[key] = nc
    return _NC_CACHE[key]


LAST_RESULTS = None


def kernel(**inputs):
    global LAST_RESULTS
    world = np.ascontiguousarray(np.asarray(inputs["world"], dtype=np.float32))
    bc = np.ascontiguousarray(np.asarray(inputs["rand_interact"], dtype=np.float32))
    re_ = np.ascontiguousarray(np.asarray(inputs["rand_element"], dtype=np.float32))
    vf = np.ascontiguousarray(np.asarray(inputs["velocity_field"], dtype=np.float32))
    mats = _build_mats()

    # Fast path: carry the (one-hot) world in bf16 on device — exact as long
    # as every world value round-trips f32->bf16->f32 unchanged.
    world_bf = world.astype(ml_dtypes.bfloat16)
    fast = bool(np.array_equal(world_bf.astype(np.float32), world))
    world_dev = world_bf if fast else world

    nc = _get_nc(B_LOC, fast)
    in_maps = []
    for i in range(NCORES):
        sl = slice(i * B_LOC, (i + 1) * B_LOC)
        in_maps.append(
            {
                "world": np.ascontiguousarray(world_dev[sl]),
                "rand_interact": np.ascontiguousarray(bc[sl]),
                "rand_element": np.ascontiguousarray(re_[sl]),
                "velocity_field": np.ascontiguousarray(vf[sl]),
                "mats": mats,
            }
        )
    res = bass_utils.run_bass_kernel_spmd(nc, in_maps, core_ids=list(range(NCORES)))
    LAST_RESULTS = res
    w4 = np.concatenate([r["w4"] for r in res.results], 0)
    if fast:
        w4 = w4.astype(np.float32)
    vfo = np.concatenate([r["vf"] for r in res.results], 0)
    return w4, vfo
